# revision 16
# baseline (speedup 1.0000x reference)
"""Trainium2 Bass kernel for nn_AttentionHead (single-head attention with RoPE + QK-norm).

Contract: kernel(**inputs) takes FULL unsharded inputs
  input_vecs [4, 2048, 1024] f32, qkv_w [3072, 1024] f32, sqk [1024] f32
and returns the FULL output [4, 2048, 1024] f32.

Sharding: 8 cores = (batch b, half z). Core (b, z) projects K/V for its half
of the sequence, exchanges K^T (with per-key 1/||K|| packed as an extra ROW of
the same gather blob -- rows stay 2048B aligned) and V via pairwise AllGather,
and computes attention for query blocks {0,3} (z=0) or {1,2} (z=1).

v3 notes:
  - one collective for K+rnk; phase order K -> Q -> V -> scores -> AV hides
    both collectives under projection/score compute.
  - rope/squares as [128,2048] DVE ops (1/4 the instruction count).
  - row norms via ones-stationary matmuls into [1,512]; the DMA access
    pattern transposes rnk rows back to per-partition layout on reload.
  - sqk^2 * sqrt(D) folded into the host-side Q rope tables.
  - V and masks SBUF-resident; softmax denominators fused into the AV
    matmuls (same stationary ex tile, one extra N=1 matmul per k-tile).
  - bf16 output, upcast on host.
"""

import numpy as np
import ml_dtypes

D = 1024          # d_model == d_internal
T = 2048
B = 4
HALF = T // 2     # 1024 tokens of K/V per core
ROPE_BASE = 10000.0
NC = 8            # cores
ND = D // 128     # 8 d-tiles
NTC = D // 128    # 8 contraction c-tiles
QB = 512          # query block size
NKLO, NKHI = 8, 16  # k-tiles processed for chunk-lo / chunk-hi (uniform program)

BF16 = ml_dtypes.bfloat16

# ---------------------------------------------------------------------------
# Infra patch: this walrus build supports only ONE sync-wait per instruction.
# Tile attaches multiple; split the extras onto NoOps inserted just before.
# ---------------------------------------------------------------------------
_PATCHED = False


def _install_patches():
    global _PATCHED
    if _PATCHED:
        return
    _PATCHED = True
    import json as _json
    import concourse.bass as _bass

    orig = _bass.Bass.to_json_bytes

    def _split_waits(m):
        ctr = 0
        for fn in m.get("functions", []):
            for blk in fn.get("blocks", []):
                insts = blk.get("instructions")
                if not insts:
                    continue
                out = []
                changed = False
                for inst in insts:
                    si = inst.get("sync_info")
                    w = (si or {}).get("on_wait") or []
                    if len(w) > 1:
                        changed = True
                        for j in range(len(w) - 1):
                            ctr += 1
                            out.append({
                                "name": f"waitsplit-{ctr}-{inst['name']}",
                                "engine": inst["engine"],
                                "opcode": "NoOp",
                                "ins": [],
                                "outs": [],
                                "sync_info": {"on_wait": [w[j]], "on_update": []},
                            })
                        si["on_wait"] = [w[-1]]
                    out.append(inst)
                if changed:
                    blk["instructions"] = out
        return m, ctr

    def to_json_bytes(self):
        raw = orig(self)
        m = _json.loads(raw)
        m, n = _split_waits(m)
        if n:
            raw = _json.dumps(m).encode()
        return raw

    _bass.Bass.to_json_bytes = to_json_bytes


# ---------------------------------------------------------------------------
# Bass program (identical for all 8 cores; per-core behavior comes from data)
# ---------------------------------------------------------------------------
_PROGRAM = None
_GROUPS = [[0, 1], [2, 3], [4, 5], [6, 7]]


def _build_program():
    import concourse.bass as bass
    import concourse.mybir as mybir
    from concourse.tile import TileContext

    BF = mybir.dt.bfloat16
    F32 = mybir.dt.float32
    AF = mybir.ActivationFunctionType
    OP = mybir.AluOpType

    nc = bass.Bass(num_devices=NC)

    # ---- I/O ----
    xh_d = nc.dram_tensor("xh", [D, HALF], BF, kind="ExternalInput")       # x^T cols of my half
    xq_d = nc.dram_tensor("xq", [D, 2 * QB], BF, kind="ExternalInput")     # x^T cols of q-blocks
    wt_d = nc.dram_tensor("wt", [D, 3 * D], BF, kind="ExternalInput")      # W^T
    cosh_d = nc.dram_tensor("cosh", [D // 2, HALF], BF, kind="ExternalInput")
    sinh_d = nc.dram_tensor("sinh", [D // 2, HALF], BF, kind="ExternalInput")
    cql_d = nc.dram_tensor("cql", [D // 2, 2 * QB], BF, kind="ExternalInput")   # s2_lo * cos
    sql_d = nc.dram_tensor("sql", [D // 2, 2 * QB], BF, kind="ExternalInput")   # s2_lo * sin
    cqh_d = nc.dram_tensor("cqh", [D // 2, 2 * QB], BF, kind="ExternalInput")   # s2_hi * cos
    sqh_d = nc.dram_tensor("sqh", [D // 2, 2 * QB], BF, kind="ExternalInput")   # s2_hi * sin
    mlo_d = nc.dram_tensor("masklo", [NKLO, 128, QB], BF, kind="ExternalInput")
    mhi_d = nc.dram_tensor("maskhi", [NKHI - NKLO, 128, QB], BF, kind="ExternalInput")
    out_d = nc.dram_tensor("out", [2 * QB, D], BF, kind="ExternalOutput")
    # collective staging. EXACTLY [D, HALF]: odd sizes make NRT pick the slow
    # Ring algorithm over Mesh. K ships pre-normalized by 1/||K|| so no extra
    # rnk payload is needed at all.
    kh_d = nc.dram_tensor("khalf", [D, HALF], BF, kind="Internal")
    kg_d = nc.dram_tensor("kgath", [2 * D, HALF], BF, kind="Internal")
    vh_d = nc.dram_tensor("vhalf", [HALF, D], BF, kind="Internal")
    vg_d = nc.dram_tensor("vgath", [T, D], BF, kind="Internal")            # global V

    def r_ipf(ap, p=128):
        return ap.rearrange("(i p) f -> p i f", p=p)

    def r_pif(ap, f):
        return ap.rearrange("p (i f) -> p i f", f=f)

    with TileContext(nc) as tc:
        with tc.tile_pool(name="persist", bufs=1) as pp:
            kt = pp.tile([128, ND * T], BF, tag="kt")             # 32K/part
            qt = pp.tile([128, 2 * ND * QB], BF, tag="qt")        # 16K  [p, ch*4096 + i*512]
            vga = pp.tile([128, 8 * D], BF, tag="vga")            # 16K (keys 0..1023)
            ones_bf = pp.tile([128, 1], BF, tag="ones_bf")
            ones1x = pp.tile([1, 128], F32, tag="ones1x")

            nc.vector.memset(ones_bf[:], 1.0)
            nc.vector.memset(ones1x[:], 1.0)

            with tc.tile_pool(name="pA", bufs=1) as pa, \
                 tc.tile_pool(name="ptrig", bufs=3) as ptrig, \
                 tc.tile_pool(name="pkm", bufs=3) as pkm, \
                 tc.tile_pool(name="pscr", bufs=1) as pscr, \
                 tc.tile_pool(name="ppbc", bufs=2) as ppbcs:
                wqk = pa.tile([128, NTC * 2 * D], BF, tag="wqk")  # 32K (W_q|W_k per c)
                xh = pa.tile([128, NTC * HALF], BF, tag="xh")     # 16K
                wv = pa.tile([128, NTC * D], BF, tag="wv")        # 16K
                xq = pa.tile([128, NTC * 2 * QB], BF, tag="xq")   # 16K
                kthc = pa.tile([128, ND * QB], BF, tag="kthc")    # 8K (one K^T chunk)

                # ---- P0 loads (sync queue), K-projection inputs first ----
                wqk3 = r_pif(wqk[:, :], 2 * D)     # [p, c, 2D]
                xh3 = r_pif(xh[:, :], HALF)
                for c0, cn in ((0, 1), (1, 1), (2, 2), (4, 2), (6, 2)):
                    nc.sync.dma_start(wqk3[:, c0:c0 + cn, D:2 * D],
                                      r_ipf(wt_d[:, D:2 * D])[:, c0:c0 + cn, :])
                    nc.sync.dma_start(xh3[:, c0:c0 + cn, :],
                                      r_ipf(xh_d[:, :])[:, c0:c0 + cn, :])
                trig = []
                for ch in range(2):
                    ct = ptrig.tile([128, 4 * QB], BF, tag="trig", name=f"cosc{ch}")
                    st = ptrig.tile([128, 4 * QB], BF, tag="trig", name=f"sinc{ch}")
                    k0 = ch * QB
                    nc.sync.dma_start(r_pif(ct[:, :], QB), r_ipf(cosh_d[:, k0:k0 + QB]))
                    nc.sync.dma_start(r_pif(st[:, :], QB), r_ipf(sinh_d[:, k0:k0 + QB]))
                    trig.append((ct, st))
                for c0 in range(0, NTC, 4):
                    nc.sync.dma_start(wqk3[:, c0:c0 + 4, 0:D],
                                      r_ipf(wt_d[:, 0:D])[:, c0:c0 + 4, :])
                    nc.sync.dma_start(r_pif(xq[:, :], 2 * QB)[:, c0:c0 + 4, :],
                                      r_ipf(xq_d[:, :])[:, c0:c0 + 4, :])
                for c0 in range(0, NTC, 4):
                    nc.sync.dma_start(r_pif(wv[:, :], D)[:, c0:c0 + 4, :],
                                      r_ipf(wt_d[:, 2 * D:3 * D])[:, c0:c0 + 4, :])

                kms = {}      # (ch, half) -> [128, 4*QB] bf16 (4 d-tiles merged)
                sqk_t = {}    # ch -> (sq_l, sq_h)
                qms = {}

                with tc.tile_pool(name="p2pn", bufs=1, space="PSUM") as pnkp, \
                     tc.tile_pool(name="pbcp", bufs=1, space="PSUM") as psbc:
                    # ---- P2: K projection (c-outer) ----
                    with tc.tile_pool(name="p2ps", bufs=6, space="PSUM") as pk:
                        for ch in range(2):
                            k0 = ch * QB
                            for g in range(2):
                                kmg = pkm.tile([128, 4 * QB], BF, tag="km", name=f"km{ch}{g}")
                                ps = {}
                                for j in range(4):
                                    ps[j] = pk.tile([128, QB], F32, tag="pk", name=f"pk{ch}{g}{j}")
                                for c in range(NTC):
                                    for j in range(4):
                                        dt = g * 4 + j
                                        nc.tensor.matmul(
                                            ps[j][:],
                                            wqk[:, c * 2 * D + D + dt * 128: c * 2 * D + D + (dt + 1) * 128],
                                            xh[:, c * HALF + k0: c * HALF + k0 + QB],
                                            start=(c == 0), stop=(c == NTC - 1))
                                for j in range(4):
                                    nc.scalar.copy(kmg[:, j * QB:(j + 1) * QB], ps[j][:])
                                kms[(ch, g)] = kmg

                    # squares + rope (DVE only; kh ships after per-key normalize)
                    def emit_k_sqrope(ch):
                        sl = pscr.tile([128, 4 * QB], BF, tag="sql", name=f"ksql{ch}")
                        sh = pscr.tile([128, 4 * QB], BF, tag="sqh", name=f"ksqh{ch}")
                        nc.vector.tensor_tensor(sl[:], kms[(ch, 0)][:], kms[(ch, 0)][:], op=OP.mult)
                        nc.vector.tensor_tensor(sh[:], kms[(ch, 1)][:], kms[(ch, 1)][:], op=OP.mult)
                        sqk_t[ch] = (sl, sh)
                        cosc, sinc = trig[ch]
                        lo, hi = kms[(ch, 0)], kms[(ch, 1)]
                        t_a = pscr.tile([128, 4 * QB], BF, tag="ra", name=f"ka{ch}")
                        t_b = pscr.tile([128, 4 * QB], BF, tag="rb", name=f"kb{ch}")
                        nc.vector.tensor_tensor(t_a[:], lo[:], cosc[:], op=OP.mult)
                        nc.vector.tensor_tensor(t_b[:], hi[:], sinc[:], op=OP.mult)
                        nc.vector.tensor_sub(kthc[:, 0:4 * QB], t_a[:], t_b[:])
                        t_c = pscr.tile([128, 4 * QB], BF, tag="ra", name=f"kc{ch}")
                        t_e = pscr.tile([128, 4 * QB], BF, tag="rb", name=f"ke{ch}")
                        nc.vector.tensor_tensor(t_c[:], hi[:], cosc[:], op=OP.mult)
                        nc.vector.tensor_tensor(t_e[:], lo[:], sinc[:], op=OP.mult)
                        nc.vector.tensor_add(kthc[:, 4 * QB:8 * QB], t_c[:], t_e[:])

                    def emit_k_pnk(ch):
                        # pnk = sum_d K^2 -> 1/||K|| as a [1, 512] row
                        pnk = pnkp.tile([1, QB], F32, tag="pnk", name=f"pnk{ch}")
                        sl, sh = sqk_t[ch]
                        for j in range(4):
                            nc.tensor.matmul(pnk[:], ones_bf[:], sl[:, j * QB:(j + 1) * QB],
                                             start=(j == 0), stop=False)
                        for j in range(4):
                            nc.tensor.matmul(pnk[:], ones_bf[:], sh[:, j * QB:(j + 1) * QB],
                                             start=False, stop=(j == 3))
                        rkq = pscr.tile([1, QB], F32, tag="rnq", name=f"rkq{ch}")
                        nc.scalar.activation(rkq[:], pnk[:], AF.Sqrt)
                        nc.vector.reciprocal(rkq[:], rkq[:])
                        return rkq

                    def emit_k_ship(ch, rkq):
                        # broadcast 1/||K|| over partitions, normalize kthc, ship
                        k0 = ch * QB
                        bcp = psbc.tile([128, QB], F32, tag="pbc", name=f"kbcp{ch}")
                        nc.tensor.matmul(bcp[:], ones1x[:], rkq[:], start=True, stop=True)
                        bcs = ppbcs.tile([128, QB], BF, tag="pbcs", name=f"kbc{ch}")
                        nc.scalar.copy(bcs[:], bcp[:])
                        bc3 = r_pif(bcs[:, :], QB).broadcast_to([128, 4, QB])
                        nc.vector.tensor_tensor(r_pif(kthc[:, 0:4 * QB], QB),
                                                r_pif(kthc[:, 0:4 * QB], QB), bc3, op=OP.mult)
                        nc.vector.tensor_tensor(r_pif(kthc[:, 4 * QB:8 * QB], QB),
                                                r_pif(kthc[:, 4 * QB:8 * QB], QB), bc3, op=OP.mult)
                        nc.scalar.dma_start(r_ipf(kh_d[0:D, k0:k0 + QB]),
                                            r_pif(kthc[:, :], QB))

                    emit_k_sqrope(0)
                    rkq0 = emit_k_pnk(0)
                    emit_k_ship(0, rkq0)
                    emit_k_sqrope(1)

                    # ---- P3: V projection (first tiles fill the pnk1 DVE wait) ----
                    with tc.tile_pool(name="p3ps", bufs=3, space="PSUM") as psv, \
                         tc.tile_pool(name="p3v", bufs=2) as pvt:

                        def v_proj(tt_range):
                            for tt in tt_range:
                                vt = pvt.tile([128, D], BF, tag="vt", name=f"vt{tt}")
                                for dch in range(2):
                                    p = psv.tile([128, 512], F32, tag="pv", name=f"pv{tt}{dch}")
                                    for c in range(NTC):
                                        nc.tensor.matmul(p[:], xh[:, c * HALF + tt * 128: c * HALF + (tt + 1) * 128],
                                                         wv[:, c * D + dch * 512: c * D + (dch + 1) * 512],
                                                         start=(c == 0), stop=(c == NTC - 1))
                                    nc.scalar.copy(vt[:, dch * 512:(dch + 1) * 512], p[:])
                                nc.scalar.dma_start(vh_d[tt * 128:(tt + 1) * 128, :], vt[:])

                        v_proj(range(0, 2))
                        rkq1 = emit_k_pnk(1)
                        emit_k_ship(1, rkq1)

                        # K blob complete: gather, then reload (gpsimd queue)
                        nc.gpsimd.collective_compute(
                            kind="AllGather", op=OP.bypass, replica_groups=_GROUPS,
                            ins=[kh_d[:, :]], outs=[kg_d[:, :]])
                        kt3 = r_pif(kt[:, :], T)           # [p, i, 2048]
                        for h in range(2):
                            khsrc = r_ipf(kg_d[h * D:(h + 1) * D, :])
                            for i0 in range(0, ND, 4):
                                nc.gpsimd.dma_start(
                                    kt3[:, i0:i0 + 4, h * HALF:(h + 1) * HALF],
                                    khsrc[:, i0:i0 + 4, :])

                        v_proj(range(2, 8))

                    nc.gpsimd.collective_compute(
                        kind="AllGather", op=OP.bypass, replica_groups=_GROUPS,
                        ins=[vh_d[:, :]], outs=[vg_d[:, :]])
                    vga3 = r_pif(vga[:, :], D)
                    vsrc = vg_d[0:HALF, :].rearrange("(k p) d -> p k d", p=128)
                    for k0 in range(0, 8, 4):
                        nc.gpsimd.dma_start(vga3[:, k0:k0 + 4, :], vsrc[:, k0:k0 + 4, :])

                    # ---- P1: Q projection (chains interleaved with K norms / CC) ----
                    with tc.tile_pool(name="p1ps", bufs=4, space="PSUM") as psq:

                        def q_chains(ch, g):
                            q0 = ch * QB
                            qmg = pkm.tile([128, 4 * QB], BF, tag="km", name=f"qm{ch}{g}")
                            for j in range(4):
                                i = g * 4 + j
                                p = psq.tile([128, QB], F32, tag="pq", name=f"pq{ch}{i}")
                                for c in range(NTC):
                                    nc.tensor.matmul(p[:], wqk[:, c * 2 * D + i * 128: c * 2 * D + (i + 1) * 128],
                                                     xq[:, c * 2 * QB + q0: c * 2 * QB + q0 + QB],
                                                     start=(c == 0), stop=(c == NTC - 1))
                                nc.scalar.copy(qmg[:, j * QB:(j + 1) * QB], p[:])
                            qms[(ch, g)] = qmg

                        q_chains(0, 0)
                        q_chains(0, 1)
                        q_chains(1, 0)

                        def q_squares(ch):
                            sl = pscr.tile([128, 4 * QB], BF, tag="sql", name=f"qsql{ch}")
                            sh = pscr.tile([128, 4 * QB], BF, tag="sqh", name=f"qsqh{ch}")
                            nc.vector.tensor_tensor(sl[:], qms[(ch, 0)][:], qms[(ch, 0)][:], op=OP.mult)
                            nc.vector.tensor_tensor(sh[:], qms[(ch, 1)][:], qms[(ch, 1)][:], op=OP.mult)
                            return sl, sh

                        with tc.tile_pool(name="p1pn", bufs=2, space="PSUM") as psnq:

                            def q_norm(ch, sl, sh):
                                pnq = psnq.tile([1, QB], F32, tag="pnq", name=f"pnq{ch}")
                                for j in range(4):
                                    nc.tensor.matmul(pnq[:], ones_bf[:], sl[:, j * QB:(j + 1) * QB],
                                                     start=(j == 0), stop=False)
                                for j in range(4):
                                    nc.tensor.matmul(pnq[:], ones_bf[:], sh[:, j * QB:(j + 1) * QB],
                                                     start=False, stop=(j == 3))
                                rnq = pscr.tile([1, QB], F32, tag="rnq", name=f"rnq{ch}")
                                nc.scalar.activation(rnq[:], pnq[:], AF.Sqrt)
                                nc.vector.reciprocal(rnq[:], rnq[:])
                                pbcp = psbc.tile([128, QB], F32, tag="pbc", name=f"pbcp{ch}")
                                nc.tensor.matmul(pbcp[:], ones1x[:], rnq[:], start=True, stop=True)
                                pbc = ppbcs.tile([128, QB], BF, tag="pbcs", name=f"pbc{ch}")
                                nc.scalar.copy(pbc[:], pbcp[:])
                                return pbc

                            sl0, sh0 = q_squares(0)
                            pbc0 = q_norm(0, sl0, sh0)
                            q_chains(1, 1)
                            sl1, sh1 = q_squares(1)
                            pbc1 = q_norm(1, sl1, sh1)

                        def q_rope(ch, pbc):
                            q0 = ch * QB
                            cql = ptrig.tile([128, 4 * QB], BF, tag="trig", name=f"cql{ch}")
                            sql = ptrig.tile([128, 4 * QB], BF, tag="trig", name=f"sql{ch}")
                            cqh = ptrig.tile([128, 4 * QB], BF, tag="trig", name=f"cqh{ch}")
                            sqh = ptrig.tile([128, 4 * QB], BF, tag="trig", name=f"sqh{ch}")
                            nc.sync.dma_start(r_pif(cql[:, :], QB), r_ipf(cql_d[:, q0:q0 + QB]))
                            nc.sync.dma_start(r_pif(sql[:, :], QB), r_ipf(sql_d[:, q0:q0 + QB]))
                            nc.sync.dma_start(r_pif(cqh[:, :], QB), r_ipf(cqh_d[:, q0:q0 + QB]))
                            nc.sync.dma_start(r_pif(sqh[:, :], QB), r_ipf(sqh_d[:, q0:q0 + QB]))
                            lo, hi = qms[(ch, 0)], qms[(ch, 1)]
                            pb3 = r_pif(pbc[:, :], QB).broadcast_to([128, 4, QB])
                            t_a = pscr.tile([128, 4 * QB], BF, tag="ra", name=f"qa{ch}")
                            t_b = pscr.tile([128, 4 * QB], BF, tag="rb", name=f"qb{ch}")
                            nc.vector.tensor_tensor(t_a[:], lo[:], cql[:], op=OP.mult)
                            nc.vector.tensor_tensor(t_b[:], hi[:], sql[:], op=OP.mult)
                            nc.vector.tensor_sub(t_a[:], t_a[:], t_b[:])
                            nc.vector.tensor_tensor(r_pif(qt[:, ch * 8 * QB: ch * 8 * QB + 4 * QB], QB),
                                                    r_pif(t_a[:, :], QB), pb3, op=OP.mult)
                            t_c = pscr.tile([128, 4 * QB], BF, tag="ra", name=f"qc{ch}")
                            t_e = pscr.tile([128, 4 * QB], BF, tag="rb", name=f"qe{ch}")
                            nc.vector.tensor_tensor(t_c[:], hi[:], cqh[:], op=OP.mult)
                            nc.vector.tensor_tensor(t_e[:], lo[:], sqh[:], op=OP.mult)
                            nc.vector.tensor_add(t_c[:], t_c[:], t_e[:])
                            nc.vector.tensor_tensor(r_pif(qt[:, ch * 8 * QB + 4 * QB: ch * 8 * QB + 8 * QB], QB),
                                                    r_pif(t_c[:, :], QB), pb3, op=OP.mult)

                        q_rope(0, pbc0)
                        q_rope(1, pbc1)


            # ---- P4: attention ----
            with tc.tile_pool(name="pC", bufs=1) as pc, \
                 tc.tile_pool(name="pCot", bufs=2) as pot, \
                 tc.tile_pool(name="pCscr", bufs=2) as pcs, \
                 tc.tile_pool(name="p4pss", bufs=2, space="PSUM") as pss, \
                 tc.tile_pool(name="p4psd", bufs=2, space="PSUM") as psd, \
                 tc.tile_pool(name="p4pso", bufs=4, space="PSUM") as pso:
                mlo = pc.tile([128, NKLO * QB], BF, tag="mlo")
                mhi = pc.tile([128, (NKHI - NKLO) * QB], BF, tag="mhi")
                exlo = pc.tile([128, NKLO * QB], BF, tag="exlo")
                exhi = pc.tile([128, NKHI * QB], BF, tag="exhi")
                vgb = pc.tile([128, 8 * D], BF, tag="vgb")        # keys 1024..2047
                nc.sync.dma_start(r_pif(mlo[:, :], QB),
                                  mlo_d[:, :, :].rearrange("j p f -> p j f"))
                nc.sync.dma_start(r_pif(mhi[:, :], QB),
                                  mhi_d[:, :, :].rearrange("j p f -> p j f"))
                vgb3 = r_pif(vgb[:, :], D)
                vsrcb = vg_d[HALF:T, :].rearrange("(k p) d -> p k d", p=128)
                for k0 in range(0, 8, 4):
                    nc.gpsimd.dma_start(vgb3[:, k0:k0 + 4, :], vsrcb[:, k0:k0 + 4, :])

                chunks = ((NKLO, exlo, 0, mlo), (NKHI, exhi, NKLO, mhi))
                # scores + exp for both chunks first, then the AV passes
                for ch, (n_k, ex, mask_start, mt) in enumerate(chunks):
                    q0 = ch * 8 * QB
                    for kti in range(n_k):
                        ps_s = pss.tile([128, QB], F32, tag="pscore", name=f"ps{ch}{kti}")
                        for i in range(ND):
                            nc.tensor.matmul(ps_s[:], kt[:, i * T + kti * 128: i * T + (kti + 1) * 128],
                                             qt[:, q0 + i * QB: q0 + (i + 1) * QB],
                                             start=(i == 0), stop=(i == ND - 1))
                        exsl = ex[:, kti * QB:(kti + 1) * QB]
                        nc.scalar.activation(exsl, ps_s[:], AF.Exp, bias=0.0)
                        if kti >= mask_start:
                            moff = (kti - mask_start) * QB
                            nc.vector.tensor_tensor(exsl, exsl, mt[:, moff:moff + QB], op=OP.mult)
                for ch, (n_k, ex, mask_start, mt) in enumerate(chunks):
                    q0c = ch * QB
                    for sub in range(4):
                        po0 = pso.tile([128, 512], F32, tag="pout", name=f"poa{ch}{sub}")
                        po1 = pso.tile([128, 512], F32, tag="pout", name=f"pob{ch}{sub}")
                        pden = psd.tile([128, 1], F32, tag="pden", name=f"pden{ch}{sub}")
                        for kti in range(n_k):
                            vsr = vga if kti < 8 else vgb
                            vcol = (kti % 8) * D
                            exs = ex[:, kti * QB + sub * 128: kti * QB + (sub + 1) * 128]
                            nc.tensor.matmul(po0[:], exs, vsr[:, vcol:vcol + 512],
                                             start=(kti == 0), stop=(kti == n_k - 1))
                            nc.tensor.matmul(po1[:], exs, vsr[:, vcol + 512:vcol + 1024],
                                             start=(kti == 0), stop=(kti == n_k - 1))
                            nc.tensor.matmul(pden[:], exs, ones_bf[:],
                                             start=(kti == 0), stop=(kti == n_k - 1))
                        rd = pcs.tile([128, 1], F32, tag="rd", name=f"rd{ch}{sub}")
                        nc.vector.reciprocal(rd[:], pden[:])
                        ot = pot.tile([128, D], BF, tag="ot", name=f"ot{ch}{sub}")
                        nc.scalar.activation(ot[:, 0:512], po0[:], AF.Copy, bias=0.0, scale=rd[:])
                        nc.scalar.activation(ot[:, 512:1024], po1[:], AF.Copy, bias=0.0, scale=rd[:])
                        nc.sync.dma_start(out_d[q0c + sub * 128: q0c + (sub + 1) * 128, :], ot[:])

    return nc


def _get_program():
    global _PROGRAM
    if _PROGRAM is None:
        _install_patches()
        _PROGRAM = _build_program()
    return _PROGRAM


# ---------------------------------------------------------------------------
# Host-side prep + launch
# ---------------------------------------------------------------------------
def _rope_tables():
    inv_freq = (1.0 / (ROPE_BASE ** (np.arange(0, D, 2, dtype=np.float32) / D))).astype(np.float32)
    t = np.arange(T, dtype=np.float32)
    freqs = t[:, None] * inv_freq[None, :]          # [T, 512]
    cos = np.cos(freqs).T.copy()                    # [512, T]
    sin = np.sin(freqs).T.copy()
    return cos, sin


def _mask_tiles(block, kt_lo, kt_hi):
    """[kt_hi-kt_lo, 128, 512] 0/1: allowed = key_global <= query_global."""
    n = kt_hi - kt_lo
    m = np.zeros((n, 128, QB), dtype=np.float32)
    qg = block * QB + np.arange(QB)[None, :]
    for idx, kti in enumerate(range(kt_lo, kt_hi)):
        kg = kti * 128 + np.arange(128)[:, None]
        m[idx] = (kg <= qg).astype(np.float32)
    return m


# kept for test.py introspection
LAST_RESULT = None


def kernel(input_vecs, qkv_w, sqk, _trace=False):
    global LAST_RESULT
    _install_patches()
    from concourse.bass_utils import run_bass_kernel_spmd

    nc = _get_program()

    f32 = np.float32
    x = np.asarray(input_vecs, f32)
    w = np.asarray(qkv_w, f32)
    s = np.asarray(sqk, f32)

    wt_bf = np.ascontiguousarray(w.T).astype(BF16)                  # [1024, 3072]
    sqk_eff = s * np.sqrt(np.float32(D)).astype(f32)
    s2 = (np.sqrt(np.float32(D)).astype(f32) * sqk_eff * sqk_eff).astype(f32)   # [1024]
    cos, sin = _rope_tables()
    cql_t = cos * s2[:512, None]
    sql_t = sin * s2[:512, None]
    cqh_t = cos * s2[512:, None]
    sqh_t = sin * s2[512:, None]

    in_maps = []
    metas = []
    for c in range(NC):
        b, z = c // 2, c % 2
        blo, bhi = (0, 3) if z == 0 else (1, 2)
        xt = np.ascontiguousarray(x[b].T)                           # [1024, 2048] f32
        qcols = np.concatenate([xt[:, blo * QB:(blo + 1) * QB],
                                xt[:, bhi * QB:(bhi + 1) * QB]], axis=1)

        def qtab(tab):
            return np.ascontiguousarray(np.concatenate(
                [tab[:, blo * QB:(blo + 1) * QB], tab[:, bhi * QB:(bhi + 1) * QB]],
                axis=1)).astype(BF16)

        h0 = z * HALF
        in_maps.append({
            "xh": np.ascontiguousarray(xt[:, h0:h0 + HALF]).astype(BF16),
            "xq": np.ascontiguousarray(qcols).astype(BF16),
            "wt": wt_bf,
            "cosh": np.ascontiguousarray(cos[:, h0:h0 + HALF]).astype(BF16),
            "sinh": np.ascontiguousarray(sin[:, h0:h0 + HALF]).astype(BF16),
            "cql": qtab(cql_t),
            "sql": qtab(sql_t),
            "cqh": qtab(cqh_t),
            "sqh": qtab(sqh_t),
            "masklo": _mask_tiles(blo, 0, NKLO).astype(BF16),
            "maskhi": _mask_tiles(bhi, NKLO, NKHI).astype(BF16),
        })
        metas.append((b, blo, bhi))

    res = run_bass_kernel_spmd(nc, in_maps, core_ids=list(range(NC)), trace=_trace)
    LAST_RESULT = res

    out = np.empty((B, T, D), dtype=f32)
    for c, (b, blo, bhi) in enumerate(metas):
        o = np.asarray(res.results[c]["out"]).astype(f32)
        out[b, blo * QB:(blo + 1) * QB] = o[:QB]
        out[b, bhi * QB:(bhi + 1) * QB] = o[QB:]
    return out
